# revision 10
# baseline (speedup 1.0000x reference)
"""Trainium2 Bass kernel for nn_Decoder: dense MLP (sigmoid) + fixed-COO sparse matmul.

Computation:
    h = sigmoid(w @ W1.T + b1)                       # [B=128, H=8192]
    out_sp[b, r] = sum_e{rows[e]==r} edge_vals[e] * h[b, cols[e]]   # [B, OUT=32768]
    out = scale * out_sp + ref

Strategy (8 NeuronCores, SPMD, row-partitioned):
  - Core k owns output rows [4096k, 4096(k+1)).
  - The sparse matrix is canonicalized host-side (COO -> dense per-core
    slice, duplicates summed) and quantized to fp8-e4m3 with the per-row
    `scale` and a 0.5 factor folded in: Sq = e4m3(0.5*scale*S).  Streaming
    the 32 MB/core fp8 slice runs at HBM rate and stage B becomes
    DoubleRow fp8 matmuls (2 contraction rows per partition) with h
    resident in SBUF as the stationary operand.
  - fp8 h error is tamed by centering: sigmoid(z) = 0.5 + 0.5*tanh(z/2),
    so the device computes t = tanh(z/2) (full (-1,1) fp8 range) and
      out = t @ Sq + REF',   REF' = ref + colsum_f32(Sq)
    which absorbs the 0.5*scale*colsum(S) DC term exactly (colsum taken
    over the *quantized* Sq so no extra mismatch).  Measured rms rel err
    ~1.6e-2 (gate 2e-2).
  - REF' is accumulated into PSUM by a K=1 ones-matmul per bank (16 KB
    HBM instead of a replicated 1 MB stream); evacuation is a plain DVE
    copy; out written fp16.
  - S streams as 1 MB slabs on the sync HWDGE ring only (out-DMAs live on
    the scalar ring so they never head-of-line-block the stream); W1
    (fp8) slice 0 leads the sync ring so stage A starts by ~3 us, the
    rest rides the scalar ring.
  - b1 is applied as the per-partition ACT bias (one tanh per hidden
    chunk), so stage A has no K=1 bias matmuls.
  - The last slab of each pass runs bank-major with per-bank stop +
    evacuation so the tail pipeline drains per bank.
"""

import numpy as np
import ml_dtypes

import concourse.bass as bass
import concourse.mybir as mybir
from concourse.tile import TileContext
from concourse.bass_utils import run_bass_kernel_spmd

LATENT, HIDDEN, OUT, BATCH = 256, 8192, 32768, 128
NCORES = 8
RPC = OUT // NCORES          # rows per core = 4096
RBLK = 512                   # output rows per PSUM bank
NRB = RPC // RBLK            # 8 row blocks per core
NPASS = 2
HRB = NRB // NPASS           # row blocks per pass
HB = HIDDEN // 128           # 64 hidden chunks
CP = HB // 2                 # 32 hidden chunk-pairs (DoubleRow k-tiles)
SLAB = 2                     # chunk-pairs per S slab (1 MB DMAs)
NSLAB = CP // SLAB           # 16 slabs per pass

_NC_CACHE = {}


def _split_multiwaits(nc):
    """walrus codegen embeds at most ONE sync wait per ISA instruction and
    errors with "Too many sync wait commands" otherwise.  Split extra waits
    into single-wait NoOps on the same engine immediately before the
    instruction (engine streams keep program order through walrus)."""
    for f in nc.m.functions:
        for bb in f.blocks:
            out, changed = [], False
            for ins in bb.instructions:
                si = ins.sync_info
                waits = list(si.on_wait) if si and si.on_wait else []
                if len(waits) > 1:
                    changed = True
                    for wsub in waits[:-1]:
                        n = mybir.InstNoOp(name=f"I-{nc.next_id()}", ins=[], outs=[])
                        n.engine = ins.engine
                        n.sync_info = mybir.SyncInfo(on_wait=[wsub], on_update=[])
                        out.append(n)
                    ins.sync_info = mybir.SyncInfo(
                        on_wait=waits[-1:], on_update=list(si.on_update or [])
                    )
                out.append(ins)
            if changed:
                bb.instructions = out


def _build_nc():
    fp32 = mybir.dt.float32
    f16 = mybir.dt.float16
    f8 = mybir.dt.float8e4
    TANH = mybir.ActivationFunctionType.Tanh
    DR = mybir.MatmulPerfMode.DoubleRow

    nc = bass.Bass("TRN2", target_bir_lowering=False, debug=False)

    d_w1t = nc.dram_tensor("w1t", [LATENT, HIDDEN], f8, kind="ExternalInput")
    d_wt = nc.dram_tensor("wt", [LATENT, BATCH], f16, kind="ExternalInput")
    d_b1h = nc.dram_tensor("b1h", [128, HB], fp32, kind="ExternalInput")
    d_s = nc.dram_tensor(
        "sdense", [NPASS, NSLAB, 128, SLAB, 2, HRB, RBLK], f8, kind="ExternalInput"
    )
    d_rfr = nc.dram_tensor("refrow", [1, RPC], f16, kind="ExternalInput")
    d_out = nc.dram_tensor("out", [BATCH, RPC], f16, kind="ExternalOutput")

    with TileContext(nc) as tc:
        with (
            tc.tile_pool(name="consts", bufs=1) as consts,
            # Deep prefetch: the S buffer keeps the DMA engines streaming
            # continuously through stage A and across the pass boundary.
            tc.tile_pool(name="sstream", bufs=19) as sstream,
            tc.tile_pool(name="work", bufs=8) as work,
        ):
            # ---------------- constant loads ----------------
            # Sync ring order: W1 slice 0 leads (stage A can start ~3us),
            # then the tiny stage-A constants, then the S stream.
            sb_w1 = consts.tile([128, 2, HIDDEN], f8)
            w1t_ap = d_w1t.ap().rearrange("(kc p) h -> p kc h", p=128)
            nc.sync.dma_start(out=sb_w1[:, :, :2048], in_=w1t_ap[:, :, :2048])
            sb_wt = consts.tile([128, 2, BATCH], f16)
            nc.sync.dma_start(
                out=sb_wt[:],
                in_=d_wt.ap().rearrange("(kc p) b -> p kc b", p=128),
            )
            sb_b1h = consts.tile([128, HB], fp32)
            nc.sync.dma_start(out=sb_b1h[:], in_=d_b1h.ap())
            # Scalar ring: REF' row, then the rest of W1.
            sb_rfr = consts.tile([1, RPC], f16)
            nc.scalar.dma_start(out=sb_rfr[:], in_=d_rfr.ap())
            for qd in range(1, 4):
                nc.scalar.dma_start(
                    out=sb_w1[:, :, qd * 2048 : (qd + 1) * 2048],
                    in_=w1t_ap[:, :, qd * 2048 : (qd + 1) * 2048],
                )
            sb_ones8 = consts.tile([1, BATCH], f8)
            nc.gpsimd.memset(sb_ones8[:], 1.0)

            # ---------------- stage A + pass-0 interleaved ----------------
            # Stage-B pass-0 slab matmuls are woven into the stage-A quad
            # loop (one slab per quad, lagging the tanh) so PE starts
            # consuming S slabs early and the S stream never throttles.
            ht_sb = consts.tile([128, HB, BATCH], f8)

            def ref_mm(ph, j, pss):
                # out[b, r] += 1 * REF'[r]: K=1 matmul carries the stop flag.
                rb = ph * HRB + j
                nc.tensor.matmul(
                    pss[j][:],
                    lhsT=sb_ones8[:],
                    rhs=sb_rfr[:, rb * RBLK : (rb + 1) * RBLK],
                    start=False,
                    stop=True,
                )

            def evac_bank(ph, j, pss):
                rb = ph * HRB + j
                ot = work.tile([128, RBLK], f16, tag="ot")
                nc.vector.tensor_scalar_add(ot[:], pss[j][:], 0.0)
                nc.scalar.dma_start(
                    out=d_out.ap()[:, rb * RBLK : (rb + 1) * RBLK], in_=ot[:]
                )

            def emit_slab(ph, s, pss):
                st = sstream.tile([128, SLAB, 2, HRB, RBLK], f8, tag="s")
                nc.sync.dma_start(out=st[:], in_=d_s.ap()[ph, s])
                if s < NSLAB - 1:
                    for c4 in range(SLAB):
                        cp = s * SLAB + c4
                        for j in range(HRB):
                            nc.tensor.matmul(
                                pss[j][:],
                                lhsT=ht_sb[:, 2 * cp : 2 * cp + 2, :],
                                rhs=st[:, c4, :, j, :],
                                start=(cp == 0),
                                stop=False,
                                perf_mode=DR,
                            )
                else:
                    # Last slab: bank-major with per-bank stop + evacuation
                    # so the tail drains bank by bank.
                    for j in range(HRB):
                        for c4 in range(SLAB):
                            cp = s * SLAB + c4
                            nc.tensor.matmul(
                                pss[j][:],
                                lhsT=ht_sb[:, 2 * cp : 2 * cp + 2, :],
                                rhs=st[:, c4, :, j, :],
                                start=False,
                                stop=False,
                                perf_mode=DR,
                            )
                        ref_mm(ph, j, pss)
                        evac_bank(ph, j, pss)

            psB_cm = tc.tile_pool(name="psB", bufs=1, space="PSUM")
            psB = psB_cm.__enter__()
            with tc.tile_pool(name="psA", bufs=4, space="PSUM") as psA:
                pss0 = [
                    psB.tile([128, RBLK], fp32, tag=f"ops{j}", name=f"p0_{j}")
                    for j in range(HRB)
                ]
                for quad in range(HB // 4):
                    ps = psA.tile([128, 512], fp32, tag="hps")
                    for i4 in range(4):
                        i = quad * 4 + i4
                        for k in range(2):
                            nc.tensor.matmul(
                                ps[:, i4 * 128 : (i4 + 1) * 128],
                                lhsT=sb_w1[:, k, i * 128 : (i + 1) * 128],
                                rhs=sb_wt[:, k, :],
                                start=(k == 0),
                                stop=(k == 1),
                            )
                        # t = tanh(0.5*z + 0.5*b1) -> fp8, centered sigmoid;
                        # b1 folded in as the per-partition ACT bias.
                        nc.scalar.activation(
                            ht_sb[:, i, :],
                            ps[:, i4 * 128 : (i4 + 1) * 128],
                            TANH,
                            bias=sb_b1h[:, i : i + 1],
                            scale=0.5,
                        )
                    if quad >= 1:
                        emit_slab(0, quad - 1, pss0)
                emit_slab(0, NSLAB - 1, pss0)

            # ---------------- stage B pass 1 ----------------
            for ph in range(1, NPASS):
                pss = [
                    psB.tile([128, RBLK], fp32, tag=f"ops{j}", name=f"ps{ph}_{j}")
                    for j in range(HRB)
                ]
                for s in range(NSLAB):
                    emit_slab(ph, s, pss)
            psB_cm.__exit__(None, None, None)

    _split_multiwaits(nc)
    return nc


def _stage_inputs(w, W1, b1, edge_vals, rows, cols, scale, ref):
    """Pure-layout host staging: transposes, COO->dense canonicalization
    (duplicates summed, scipy-style), fp8 packing with scale and the 0.5
    sigmoid-centering factor folded in, colsum -> REF'."""
    f32 = np.float32
    f16 = np.float16
    f8 = ml_dtypes.float8_e4m3
    w = np.asarray(w, dtype=f32)
    W1 = np.asarray(W1, dtype=f32)
    b1 = np.asarray(b1, dtype=f32)
    edge_vals = np.asarray(edge_vals, dtype=f32)
    rows = np.asarray(rows, dtype=np.int64)
    cols = np.asarray(cols, dtype=np.int64)
    scale = np.asarray(scale, dtype=f32)
    ref = np.asarray(ref, dtype=f32)

    w1t = np.ascontiguousarray(W1.T.astype(f8))          # [LATENT, HIDDEN]
    wt = np.ascontiguousarray(w.T.astype(f16))           # [LATENT, BATCH]
    # ACT bias layout: b1h[p, c] = 0.5 * b1[128*c + p]
    b1h = np.ascontiguousarray((0.5 * b1).reshape(HB, 128).T.astype(f32))

    in_maps = []
    for k in range(NCORES):
        lo, hi = k * RPC, (k + 1) * RPC
        sel = (rows >= lo) & (rows < hi)
        r_k = rows[sel] - lo
        c_k = cols[sel]
        v_k = edge_vals[sel]

        # Dense per-core slice S[c, r], duplicate (c, r) entries summed,
        # then Sq = e4m3(0.5 * scale[r] * S[c, r]).
        sdense = np.zeros((HIDDEN, RPC), dtype=f32)
        np.add.at(sdense, (c_k, r_k), v_k)
        sdense *= 0.5 * scale[lo:hi][None, :]
        sq = sdense.astype(f8)
        # REF' = ref + colsum of the quantized Sq (absorbs the DC term)
        refp = ref[lo:hi] + sq.astype(f32).sum(axis=0)
        # [NPASS, NSLAB, 128, SLAB, 2, HRB, RBLK]:
        # hidden = ((s*SLAB + c4)*2 + kt)*128 + p
        sq = sq.reshape(NSLAB, SLAB, 2, 128, NPASS, HRB, RBLK).transpose(
            4, 0, 3, 1, 2, 5, 6
        )
        sq = np.ascontiguousarray(sq)

        in_maps.append(
            {
                "w1t": w1t,
                "wt": wt,
                "b1h": b1h,
                "sdense": sq,
                "refrow": np.ascontiguousarray(refp.astype(f16)[None, :]),
            }
        )
    return in_maps


def kernel(w, W1, b1, edge_vals, rows, cols, scale, ref):
    in_maps = _stage_inputs(w, W1, b1, edge_vals, rows, cols, scale, ref)
    if "nc" not in _NC_CACHE:
        _NC_CACHE["nc"] = _build_nc()
    nc = _NC_CACHE["nc"]
    res = run_bass_kernel_spmd(nc, in_maps, core_ids=list(range(NCORES)))
    out = np.concatenate([r["out"] for r in res.results], axis=1)
    return out.astype(np.float32)


if __name__ == "__main__":
    rng = np.random.default_rng(0)
    nnz = OUT * 32
    ins = {
        "w": rng.standard_normal((BATCH, LATENT), dtype=np.float32),
        "W1": rng.standard_normal((HIDDEN, LATENT), dtype=np.float32),
        "b1": rng.standard_normal(HIDDEN, dtype=np.float32) * 0.01,
        "edge_vals": rng.standard_normal(nnz, dtype=np.float32),
        "rows": np.repeat(np.arange(OUT, dtype=np.int64), 32),
        "cols": rng.integers(0, HIDDEN, nnz).astype(np.int64),
        "scale": rng.random(OUT, dtype=np.float32) + 0.5,
        "ref": rng.standard_normal(OUT, dtype=np.float32),
    }
    out = kernel(**ins)
    print(out.shape, out.dtype)
